# revision 26
# baseline (speedup 1.0000x reference)
"""Multi-head self-attention (2D RoPE) Trainium2 Bass kernel.

Problem: x[4,512,64,64], w_qkv[1536,512], w_proj[512,512], 8 heads, hd=64,
N=4096 positions.  out = proj(attn(rope(q), rope(k)) @ v).

Sharding (8 cores): core c -> batch b=c//2, head-group g=c%2 (heads 4g..4g+3).
Each core computes a partial projection output [512, 4096] over its 256
attention-output channels; host sums the two partials per batch (the
"all-reduce" of the tensor-parallel split) and reshapes.

Per-core kernel design:
 - QKV projection as matmul with host-transposed weights; the RoPE rotation is
   linear in q, so the rotated-pair permutation J is folded into extra weight
   columns (Jq = J@Wq, Jk = J@Wk) and RoPE becomes q*COS + (Jq x)*SIN -- three
   full-width [128, n] vector ops, no per-head slicing.
 - Attention in transposed layout: S^T[m,n] = k_m . q_n via lhsT=k^T (64
   contract rows), two heads packed in the PE array via tile_position rows
   (0,0)/(64,0).  exp() runs on the scalar engine straight out of PSUM with the
   1/8 scale folded in; no max-subtraction (|S|/8 <= ~10, fp32 exp is safe).
 - P^T needs no transpose for the PV matmul (contract dim m is already on
   partitions), and V is produced directly in [m, d] layout by an extra
   matmul X^T @ Wv^T.  A ones-column appended to V makes the PV matmul emit
   the softmax denominators as output row 64 for free.
 - The whole datapath runs fp16 (inputs cast on host): fp16 streams the PE
   at full 2.4 GHz where fp32/fp32r run at half rate, and all accumulation
   stays fp32 in PSUM, so the end-to-end error remains ~1e-3.
 - q/k tiles are double-buffered across the two head pairs so pair 1's
   projection overlaps pair 0's attention (it gets a 2-bank psum aux pool
   disjoint from attention's 6 banks); the output projection runs inside
   pair 1's attention loop per 512-column chunk.
"""

import numpy as np

import concourse.bass as bass
import concourse.mybir as mybir
import concourse.tile as tile
from concourse import bacc
from concourse.bass import ts
from concourse.bass_utils import run_bass_kernel_spmd

F32 = mybir.dt.float32
F32R = mybir.dt.float32r
BF16 = mybir.dt.bfloat16
FP16 = mybir.dt.float16
AF = mybir.ActivationFunctionType

B, DIM, H, W = 4, 512, 64, 64
HEADS = 8
HD = 64
MAX_FREQ = 10000.0
N_CORES = 8

FULL = dict(N=4096, CH=512, NS=512)


def r(ap):
    return ap.bitcast(F32R)


def build_nc(N=4096, CH=512, NS=512):
    """Build the per-core Bass program (identical on all 8 cores)."""
    NMT = N // 128        # m tiles
    NCH = N // CH         # phase-1 chunks
    NNS = N // NS         # phase-3 chunks per head
    KC = DIM // 128       # contract tiles for qkv proj

    nc = bacc.Bacc("TRN2", target_bir_lowering=False, debug=False,
                   num_devices=N_CORES)

    x_d = nc.dram_tensor("x", [DIM, N], FP16, kind="ExternalInput").ap()
    wqkv_d = nc.dram_tensor("wqkvT", [DIM, 1280], FP16, kind="ExternalInput").ap()
    wv_d = nc.dram_tensor("wvT", [DIM, 260], FP16, kind="ExternalInput").ap()
    wp_d = nc.dram_tensor("wprojT", [256, DIM], FP16, kind="ExternalInput").ap()
    cos_d = nc.dram_tensor("cos", [128, N], F32, kind="ExternalInput").ap()
    sin_d = nc.dram_tensor("sin", [128, N], F32, kind="ExternalInput").ap()
    out_d = nc.dram_tensor("out", [DIM, N], F32, kind="ExternalOutput").ap()

    with tile.TileContext(nc) as tc:
        with (
            tc.tile_pool(name="singles", bufs=1) as singles,
            tc.tile_pool(name="qkpool", bufs=2) as qkpool,
            tc.tile_pool(name="xp", bufs=2) as xp,
            tc.tile_pool(name="csp", bufs=2) as csp,
            tc.tile_pool(name="ropep", bufs=2) as ropep,
            tc.tile_pool(name="ptp", bufs=6) as ptp,
            tc.tile_pool(name="nsm", bufs=2) as nsm,
            tc.tile_pool(name="ocp", bufs=4) as ocp,
            tc.tile_pool(name="osb", bufs=2) as osb,
        ):
            wq_sb = singles.tile([128, KC, 1280], FP16, tag="wq")
            for kc in range(KC):
                nc.sync.dma_start(
                    out=wq_sb[:, kc, :],
                    in_=wqkv_d[ts(kc, 128), :])
            wv_sb = singles.tile([128, KC, 260], FP16, tag="wv")
            nc.sync.dma_start(
                out=wv_sb[:],
                in_=wv_d.rearrange("(kc p) m -> p kc m", p=128))
            wp_sb = singles.tile([128, 2, DIM], FP16, tag="wp")
            nc.sync.dma_start(
                out=wp_sb[:],
                in_=wp_d.rearrange("(ct p) m -> p ct m", p=128))

            v_sb = singles.tile([128, NMT, 4, 65], FP16, tag="v_sb")
            outT = singles.tile([128, 2, N], FP16, tag="outT")

            def phase1(p, pool, vpool):
                q_rot = qkpool.tile([128, N], FP16, tag="q_rot")
                k_rot = qkpool.tile([128, N], FP16, tag="k_rot")
                for ci in range(NCH):
                    c0 = ci * CH
                    x_t = xp.tile([128, KC, CH], FP16, tag="x_t")
                    nc.sync.dma_start(
                        out=x_t[:],
                        in_=x_d[:, c0:c0 + CH].rearrange(
                            "(kc p) n -> p kc n", p=128))
                    cos_t = csp.tile([128, CH], F32, tag="cos_t")
                    nc.sync.dma_start(out=cos_t[:], in_=cos_d[:, c0:c0 + CH])
                    sin_t = csp.tile([128, CH], F32, tag="sin_t")
                    nc.sync.dma_start(out=sin_t[:], in_=sin_d[:, c0:c0 + CH])

                    mo_ps = []
                    for mo in range(4):  # q, Jq, k, Jk
                        ps = pool.tile([128, CH], F32, tag="aux")
                        col = p * 640 + mo * 128
                        for kc in range(KC):
                            nc.tensor.matmul(
                                ps[:],
                                lhsT=wq_sb[:, kc, col:col + 128],
                                rhs=x_t[:, kc, :],
                                start=(kc == 0), stop=(kc == KC - 1))
                        mo_ps.append(ps)

                    # rope: rot = pre * COS + (J pre) * SIN
                    for (a_ps, b_ps, dst) in (
                        (mo_ps[0], mo_ps[1], q_rot),
                        (mo_ps[2], mo_ps[3], k_rot),
                    ):
                        t1 = ropep.tile([128, CH], F32, tag="t1")
                        nc.vector.tensor_mul(t1[:], a_ps[:], cos_t[:])
                        t2 = ropep.tile([128, CH], F32, tag="t2")
                        nc.vector.tensor_mul(t2[:], b_ps[:], sin_t[:])
                        nc.vector.tensor_add(dst[:, c0:c0 + CH], t1[:], t2[:])

                    if p == 0:
                        # V (all 4 heads) in [m, d] layout: X^T @ Wv^T
                        for j in range(CH // 128):
                            mt = (c0 // 128) + j
                            vp = vpool.tile([128, 4, 65], F32, tag="vp")
                            for kc in range(KC):
                                nc.tensor.matmul(
                                    vp[:],
                                    lhsT=x_t[:, kc, ts(j, 128)],
                                    rhs=wv_sb[:, kc, :],
                                    start=(kc == 0), stop=(kc == KC - 1))
                            nc.vector.tensor_copy(v_sb[:, mt, :, :], vp[:])
                            nc.vector.memset(v_sb[:, mt, :, 64:65], 1.0)
                return q_rot, k_rot

            def proj_chunk(n0, aux):
                # output projection for one finished 512-column chunk
                for po in range(4):
                    pp = aux.tile([128, NS], F32, tag="aux")
                    for ct in range(2):
                        nc.tensor.matmul(
                            pp[:],
                            lhsT=wp_sb[:, ct, ts(po, 128)],
                            rhs=outT[:, ct, n0:n0 + NS],
                            start=(ct == 0), stop=(ct == 1))
                    ot = osb.tile([128, NS], F32, tag="ot")
                    nc.vector.tensor_copy(ot[:], pp[:])
                    nc.sync.dma_start(
                        out=out_d[ts(po, 128), n0:n0 + NS], in_=ot[:])

            def phase3(p, q_rot, k_rot, sp, oap, aux):
                LAG = min(3, NMT - 1)

                def emit_norm(pend):
                    ocs, pn0 = pend
                    for (oc, row0) in zip(ocs, (0, 64)):
                        rec = nsm.tile([1, NS], F32, tag="rec")
                        nc.vector.reciprocal(rec[:], oc[64:65, :])
                        rb = nsm.tile([64, NS], F32, tag="rb")
                        nc.gpsimd.partition_broadcast(rb[:], rec[:])
                        nc.vector.tensor_mul(
                            outT[row0:row0 + 64, p, pn0:pn0 + NS],
                            oc[0:64, :], rb[:])

                for ns in range(NNS):
                    n0 = ns * NS
                    oa = oap.tile([65, NS], F32, tag="oa")
                    ob = oap.tile([65, NS], F32, tag="ob")
                    # software-pipelined m-loop: QK/exp run LAG tiles ahead of
                    # PV so the PE FIFO never blocks on the accumulator banks
                    # at a chunk boundary
                    pts = {}
                    for mt in range(NMT + LAG):
                        if mt < NMT:
                            s_t = sp.tile([128, 2 * NS], F32, tag="s_t")
                            nc.tensor.matmul(
                                s_t[:, 0:NS],
                                lhsT=k_rot[0:64, ts(mt, 128)],
                                rhs=q_rot[0:64, n0:n0 + NS],
                                start=True, stop=True, tile_position=(0, 0))
                            nc.tensor.matmul(
                                s_t[:, NS:2 * NS],
                                lhsT=k_rot[64:128, ts(mt, 128)],
                                rhs=q_rot[64:128, n0:n0 + NS],
                                start=True, stop=True, tile_position=(64, 0))
                            p_t = ptp.tile([128, 2 * NS], FP16, tag="p_t")
                            nc.scalar.activation(p_t[:], s_t[:], AF.Exp,
                                                 scale=float(HD) ** -0.5)
                            pts[mt] = p_t
                        if mt >= LAG:
                            mv = mt - LAG
                            p_t = pts.pop(mv)
                            nc.tensor.matmul(
                                oa[:], lhsT=v_sb[:, mv, 2 * p + 0, :],
                                rhs=p_t[:, 0:NS],
                                start=(mv == 0), stop=(mv == NMT - 1))
                            nc.tensor.matmul(
                                ob[:], lhsT=v_sb[:, mv, 2 * p + 1, :],
                                rhs=p_t[:, NS:2 * NS],
                                start=(mv == 0), stop=(mv == NMT - 1))
                    # drain the psum accumulators with two quick copies
                    # (frees the banks for the next chunk's PV), then
                    # normalize from sbuf
                    ocs = []
                    for acc in (oa, ob):
                        oc = ocp.tile([65, NS], F32, tag="oc")
                        nc.vector.tensor_copy(oc[:], acc[:])
                        ocs.append(oc)
                    emit_norm((ocs, n0))

                if p == 1:
                    for ns in range(NNS):
                        proj_chunk(ns * NS, aux)

            # pair 0 projection gets the whole psum to itself (program head)
            with (
                tc.tile_pool(name="pps0", bufs=6, space="PSUM") as pps0,
                tc.tile_pool(name="vps0", bufs=2, space="PSUM") as vps0,
            ):
                q0, k0 = phase1(0, pps0, vps0)

            # remainder: attention pools (6 banks) + 2-bank aux shared by
            # pair-1 projection and the output projection
            with (
                tc.tile_pool(name="sp", bufs=2, space="PSUM") as sp,
                tc.tile_pool(name="oap", bufs=1, space="PSUM") as oap,
                tc.tile_pool(name="aux", bufs=2, space="PSUM") as aux,
            ):
                phase3(0, q0, k0, sp, oap, aux)
                q1, k1 = phase1(1, aux, None)
                phase3(1, q1, k1, sp, oap, aux)

    nc.compile()
    return nc


def rope_tables(h, w, n):
    """cos/sin lookup tables, tiled x4 along partitions -> [128, n]."""
    quarter = HD // 4  # 16
    pos_h, pos_w = np.meshgrid(np.arange(h, dtype=np.float64),
                               np.arange(w, dtype=np.float64), indexing="ij")
    pos = np.stack([pos_h.ravel(), pos_w.ravel()], axis=-1)[:n]
    freqs = 1.0 / (MAX_FREQ ** (np.arange(quarter, dtype=np.float64) / quarter))
    ang = np.concatenate([pos[:, 0:1] * freqs, pos[:, 1:2] * freqs], axis=-1)
    cos = np.cos(ang).T.astype(np.float32)  # [32, n]
    sin = np.sin(ang).T.astype(np.float32)
    return np.tile(cos, (4, 1)), np.tile(sin, (4, 1))


def host_prep(x, w_qkv, w_proj, n=4096, h=H, w=W):
    """Build the 8 per-core input maps."""
    x = np.asarray(x, dtype=np.float32)
    w_qkv = np.asarray(w_qkv, dtype=np.float32)
    w_proj = np.asarray(w_proj, dtype=np.float32)
    dim = x.shape[1]
    cos128, sin128 = rope_tables(h, w, n)

    def jmat(wh):  # wh [64, dim] -> J @ wh
        return np.concatenate([-wh[32:64], wh[0:32]], axis=0)

    in_maps = []
    for c in range(N_CORES):
        b, g = c // 2, c % 2
        hs = [4 * g + i for i in range(4)]
        cols = []
        for pair in range(2):
            h0, h1 = hs[2 * pair], hs[2 * pair + 1]
            wq0, wq1 = w_qkv[64 * h0:64 * h0 + 64], w_qkv[64 * h1:64 * h1 + 64]
            wk0 = w_qkv[dim + 64 * h0: dim + 64 * h0 + 64]
            wk1 = w_qkv[dim + 64 * h1: dim + 64 * h1 + 64]
            cols += [wq0, wq1, jmat(wq0), jmat(wq1),
                     wk0, wk1, jmat(wk0), jmat(wk1),
                     np.zeros((128, dim), np.float32)]  # v slot unused
        wqkvT = np.concatenate(cols, axis=0).T.copy()  # [dim, 1280]

        wvT = np.zeros((dim, 260), np.float32)
        for i, hh in enumerate(hs):
            wvT[:, 65 * i:65 * i + 64] = w_qkv[2 * dim + 64 * hh:
                                               2 * dim + 64 * hh + 64].T
        wprojT = w_proj[:, 256 * g:256 * g + 256].T.copy()  # [256, dim]

        in_maps.append({
            "x": np.ascontiguousarray(x[b].reshape(dim, n)).astype(np.float16),
            "wqkvT": np.ascontiguousarray(wqkvT).astype(np.float16),
            "wvT": wvT.astype(np.float16),
            "wprojT": np.ascontiguousarray(wprojT).astype(np.float16),
            "cos": cos128[:, :n].copy(),
            "sin": sin128[:, :n].copy(),
        })
    return in_maps


_NC_CACHE = {}


def kernel(x, w_qkv, w_proj, trace=False):
    key = "full"
    if key not in _NC_CACHE:
        _NC_CACHE[key] = build_nc(**FULL)
    nc = _NC_CACHE[key]
    in_maps = host_prep(x, w_qkv, w_proj)
    res = run_bass_kernel_spmd(nc, in_maps, list(range(N_CORES)), trace=trace)
    outs = [res.results[c]["out"] for c in range(N_CORES)]
    full = np.empty((B, DIM, H, W), np.float32)
    for b in range(B):
        full[b] = (outs[2 * b] + outs[2 * b + 1]).reshape(DIM, H, W)
    kernel.last_results = res
    return full


# revision 27
# speedup vs baseline: 1.0109x; 1.0109x over previous
"""Multi-head self-attention (2D RoPE) Trainium2 Bass kernel.

Problem: x[4,512,64,64], w_qkv[1536,512], w_proj[512,512], 8 heads, hd=64,
N=4096 positions.  out = proj(attn(rope(q), rope(k)) @ v).

Sharding (8 cores): core c -> batch b=c//2, head-group g=c%2 (heads 4g..4g+3).
Each core computes a partial projection output [512, 4096] over its 256
attention-output channels; host sums the two partials per batch (the
"all-reduce" of the tensor-parallel split) and reshapes.

Per-core kernel design:
 - QKV projection as matmul with host-transposed weights; the RoPE rotation is
   linear in q, so the rotated-pair permutation J is folded into extra weight
   columns (Jq = J@Wq, Jk = J@Wk) and RoPE becomes q*COS + (Jq x)*SIN -- three
   full-width [128, n] vector ops, no per-head slicing.
 - Attention in transposed layout: S^T[m,n] = k_m . q_n via lhsT=k^T (64
   contract rows), two heads packed in the PE array via tile_position rows
   (0,0)/(64,0).  exp() runs on the scalar engine straight out of PSUM with the
   1/8 scale folded in; no max-subtraction (|S|/8 <= ~10, fp32 exp is safe).
 - P^T needs no transpose for the PV matmul (contract dim m is already on
   partitions), and V is produced directly in [m, d] layout by an extra
   matmul X^T @ Wv^T.  A ones-column appended to V makes the PV matmul emit
   the softmax denominators as output row 64 for free.
 - The whole datapath runs fp16 (inputs cast on host): fp16 streams the PE
   at full 2.4 GHz where fp32/fp32r run at half rate, and all accumulation
   stays fp32 in PSUM, so the end-to-end error remains ~1e-3.
 - q/k tiles are double-buffered across the two head pairs so pair 1's
   projection overlaps pair 0's attention (it gets a 2-bank psum aux pool
   disjoint from attention's 6 banks); the output projection runs inside
   pair 1's attention loop per 512-column chunk.
"""

import numpy as np

import concourse.bass as bass
import concourse.mybir as mybir
import concourse.tile as tile
from concourse import bacc
from concourse.bass import ts
from concourse.bass_utils import run_bass_kernel_spmd

F32 = mybir.dt.float32
F32R = mybir.dt.float32r
BF16 = mybir.dt.bfloat16
FP16 = mybir.dt.float16
AF = mybir.ActivationFunctionType

B, DIM, H, W = 4, 512, 64, 64
HEADS = 8
HD = 64
MAX_FREQ = 10000.0
N_CORES = 8

FULL = dict(N=4096, CH=512, NS=512)


def r(ap):
    return ap.bitcast(F32R)


def build_nc(N=4096, CH=512, NS=512):
    """Build the per-core Bass program (identical on all 8 cores)."""
    NMT = N // 128        # m tiles
    NCH = N // CH         # phase-1 chunks
    NNS = N // NS         # phase-3 chunks per head
    KC = DIM // 128       # contract tiles for qkv proj

    nc = bacc.Bacc("TRN2", target_bir_lowering=False, debug=False,
                   num_devices=N_CORES)

    x_d = nc.dram_tensor("x", [DIM, N], FP16, kind="ExternalInput").ap()
    wqkv_d = nc.dram_tensor("wqkvT", [DIM, 1280], FP16, kind="ExternalInput").ap()
    wv_d = nc.dram_tensor("wvT", [DIM, 260], FP16, kind="ExternalInput").ap()
    wp_d = nc.dram_tensor("wprojT", [256, DIM], FP16, kind="ExternalInput").ap()
    cos_d = nc.dram_tensor("cos", [128, N], F32, kind="ExternalInput").ap()
    sin_d = nc.dram_tensor("sin", [128, N], F32, kind="ExternalInput").ap()
    out_d = nc.dram_tensor("out", [DIM, N], F32, kind="ExternalOutput").ap()

    with tile.TileContext(nc) as tc:
        with (
            tc.tile_pool(name="singles", bufs=1) as singles,
            tc.tile_pool(name="qkpool", bufs=2) as qkpool,
            tc.tile_pool(name="xp", bufs=2) as xp,
            tc.tile_pool(name="csp", bufs=2) as csp,
            tc.tile_pool(name="ropep", bufs=2) as ropep,
            tc.tile_pool(name="ptp", bufs=6) as ptp,
            tc.tile_pool(name="nsm", bufs=2) as nsm,
            tc.tile_pool(name="ocp", bufs=4) as ocp,
            tc.tile_pool(name="osb", bufs=2) as osb,
        ):
            wq_sb = singles.tile([128, KC, 1280], FP16, tag="wq")
            for kc in range(KC):
                nc.sync.dma_start(
                    out=wq_sb[:, kc, :],
                    in_=wqkv_d[ts(kc, 128), :])
            wv_sb = singles.tile([128, KC, 260], FP16, tag="wv")
            nc.sync.dma_start(
                out=wv_sb[:],
                in_=wv_d.rearrange("(kc p) m -> p kc m", p=128))
            wp_sb = singles.tile([128, 2, DIM], FP16, tag="wp")
            nc.sync.dma_start(
                out=wp_sb[:],
                in_=wp_d.rearrange("(ct p) m -> p ct m", p=128))

            v_sb = singles.tile([128, NMT, 4, 65], FP16, tag="v_sb")
            outT = singles.tile([128, 2, N], FP16, tag="outT")

            def phase1(p, pool, vpool):
                q_rot = qkpool.tile([128, N], FP16, tag="q_rot")
                k_rot = qkpool.tile([128, N], FP16, tag="k_rot")
                for ci in range(NCH):
                    c0 = ci * CH
                    x_t = xp.tile([128, KC, CH], FP16, tag="x_t")
                    nc.sync.dma_start(
                        out=x_t[:],
                        in_=x_d[:, c0:c0 + CH].rearrange(
                            "(kc p) n -> p kc n", p=128))
                    cos_t = csp.tile([128, CH], F32, tag="cos_t")
                    nc.sync.dma_start(out=cos_t[:], in_=cos_d[:, c0:c0 + CH])
                    sin_t = csp.tile([128, CH], F32, tag="sin_t")
                    nc.sync.dma_start(out=sin_t[:], in_=sin_d[:, c0:c0 + CH])

                    mo_ps = []
                    for mo in range(4):  # q, Jq, k, Jk
                        ps = pool.tile([128, CH], F32, tag="aux")
                        col = p * 640 + mo * 128
                        for kc in range(KC):
                            nc.tensor.matmul(
                                ps[:],
                                lhsT=wq_sb[:, kc, col:col + 128],
                                rhs=x_t[:, kc, :],
                                start=(kc == 0), stop=(kc == KC - 1))
                        mo_ps.append(ps)

                    # rope: rot = pre * COS + (J pre) * SIN
                    for (a_ps, b_ps, dst) in (
                        (mo_ps[0], mo_ps[1], q_rot),
                        (mo_ps[2], mo_ps[3], k_rot),
                    ):
                        t1 = ropep.tile([128, CH], F32, tag="t1")
                        nc.vector.tensor_mul(t1[:], a_ps[:], cos_t[:])
                        t2 = ropep.tile([128, CH], F32, tag="t2")
                        nc.vector.tensor_mul(t2[:], b_ps[:], sin_t[:])
                        nc.vector.tensor_add(dst[:, c0:c0 + CH], t1[:], t2[:])

                    if p == 0:
                        # V (all 4 heads) in [m, d] layout: X^T @ Wv^T
                        for j in range(CH // 128):
                            mt = (c0 // 128) + j
                            vp = vpool.tile([128, 4, 65], F32, tag="vp")
                            for kc in range(KC):
                                nc.tensor.matmul(
                                    vp[:],
                                    lhsT=x_t[:, kc, ts(j, 128)],
                                    rhs=wv_sb[:, kc, :],
                                    start=(kc == 0), stop=(kc == KC - 1))
                            nc.vector.tensor_copy(v_sb[:, mt, :, :], vp[:])
                            nc.vector.memset(v_sb[:, mt, :, 64:65], 1.0)
                return q_rot, k_rot

            def proj_po(n0, po, aux):
                pp = aux.tile([128, NS], F32, tag="aux")
                for ct in range(2):
                    nc.tensor.matmul(
                        pp[:],
                        lhsT=wp_sb[:, ct, ts(po, 128)],
                        rhs=outT[:, ct, n0:n0 + NS],
                        start=(ct == 0), stop=(ct == 1))
                ot = osb.tile([128, NS], F32, tag="ot")
                nc.vector.tensor_copy(ot[:], pp[:])
                nc.sync.dma_start(out=out_d[ts(po, 128), n0:n0 + NS], in_=ot[:])

            def proj_chunk(n0, aux):
                # output projection for one finished 512-column chunk
                for po in range(4):
                    pp = aux.tile([128, NS], F32, tag="aux")
                    for ct in range(2):
                        nc.tensor.matmul(
                            pp[:],
                            lhsT=wp_sb[:, ct, ts(po, 128)],
                            rhs=outT[:, ct, n0:n0 + NS],
                            start=(ct == 0), stop=(ct == 1))
                    ot = osb.tile([128, NS], F32, tag="ot")
                    nc.vector.tensor_copy(ot[:], pp[:])
                    nc.sync.dma_start(
                        out=out_d[ts(po, 128), n0:n0 + NS], in_=ot[:])

            def phase3(p, q_rot, k_rot, sp, oap, aux):
                LAG = min(3, NMT - 1)

                def emit_norm(pend):
                    ocs, pn0 = pend
                    for (oc, row0) in zip(ocs, (0, 64)):
                        rec = nsm.tile([1, NS], F32, tag="rec")
                        nc.vector.reciprocal(rec[:], oc[64:65, :])
                        rb = nsm.tile([64, NS], F32, tag="rb")
                        nc.gpsimd.partition_broadcast(rb[:], rec[:])
                        nc.vector.tensor_mul(
                            outT[row0:row0 + 64, p, pn0:pn0 + NS],
                            oc[0:64, :], rb[:])

                for ns in range(NNS):
                    n0 = ns * NS
                    oa = oap.tile([65, NS], F32, tag="oa")
                    ob = oap.tile([65, NS], F32, tag="ob")
                    # software-pipelined m-loop: QK/exp run LAG tiles ahead of
                    # PV so the PE FIFO never blocks on the accumulator banks
                    # at a chunk boundary
                    pts = {}
                    for mt in range(NMT + LAG):
                        if mt < NMT:
                            s_t = sp.tile([128, 2 * NS], F32, tag="s_t")
                            nc.tensor.matmul(
                                s_t[:, 0:NS],
                                lhsT=k_rot[0:64, ts(mt, 128)],
                                rhs=q_rot[0:64, n0:n0 + NS],
                                start=True, stop=True, tile_position=(0, 0))
                            nc.tensor.matmul(
                                s_t[:, NS:2 * NS],
                                lhsT=k_rot[64:128, ts(mt, 128)],
                                rhs=q_rot[64:128, n0:n0 + NS],
                                start=True, stop=True, tile_position=(64, 0))
                            p_t = ptp.tile([128, 2 * NS], FP16, tag="p_t")
                            nc.scalar.activation(p_t[:], s_t[:], AF.Exp,
                                                 scale=float(HD) ** -0.5)
                            pts[mt] = p_t
                        if p == 1 and ns > 0 and NMT >= 32 and \
                                mt in (12, 17, 22, 27):
                            # previous chunk's projection, spread through the
                            # m-loop so it fills PE slack instead of stalling
                            # the scalar engine at the chunk boundary
                            proj_po(n0 - NS, (mt - 12) // 5, aux)
                        if mt >= LAG:
                            mv = mt - LAG
                            p_t = pts.pop(mv)
                            nc.tensor.matmul(
                                oa[:], lhsT=v_sb[:, mv, 2 * p + 0, :],
                                rhs=p_t[:, 0:NS],
                                start=(mv == 0), stop=(mv == NMT - 1))
                            nc.tensor.matmul(
                                ob[:], lhsT=v_sb[:, mv, 2 * p + 1, :],
                                rhs=p_t[:, NS:2 * NS],
                                start=(mv == 0), stop=(mv == NMT - 1))
                    # drain the psum accumulators with two quick copies
                    # (frees the banks for the next chunk's PV), then
                    # normalize from sbuf
                    ocs = []
                    for acc in (oa, ob):
                        oc = ocp.tile([65, NS], F32, tag="oc")
                        nc.vector.tensor_copy(oc[:], acc[:])
                        ocs.append(oc)
                    emit_norm((ocs, n0))

                if p == 1:
                    if NMT >= 32:
                        proj_chunk((NNS - 1) * NS, aux)
                    else:
                        for ns in range(NNS):
                            proj_chunk(ns * NS, aux)

            # pair 0 projection gets the whole psum to itself (program head)
            with (
                tc.tile_pool(name="pps0", bufs=6, space="PSUM") as pps0,
                tc.tile_pool(name="vps0", bufs=2, space="PSUM") as vps0,
            ):
                q0, k0 = phase1(0, pps0, vps0)

            # remainder: attention pools (6 banks) + 2-bank aux shared by
            # pair-1 projection and the output projection
            with (
                tc.tile_pool(name="sp", bufs=2, space="PSUM") as sp,
                tc.tile_pool(name="oap", bufs=1, space="PSUM") as oap,
                tc.tile_pool(name="aux", bufs=2, space="PSUM") as aux,
            ):
                phase3(0, q0, k0, sp, oap, aux)
                q1, k1 = phase1(1, aux, None)
                phase3(1, q1, k1, sp, oap, aux)

    nc.compile()
    return nc


def rope_tables(h, w, n):
    """cos/sin lookup tables, tiled x4 along partitions -> [128, n]."""
    quarter = HD // 4  # 16
    pos_h, pos_w = np.meshgrid(np.arange(h, dtype=np.float64),
                               np.arange(w, dtype=np.float64), indexing="ij")
    pos = np.stack([pos_h.ravel(), pos_w.ravel()], axis=-1)[:n]
    freqs = 1.0 / (MAX_FREQ ** (np.arange(quarter, dtype=np.float64) / quarter))
    ang = np.concatenate([pos[:, 0:1] * freqs, pos[:, 1:2] * freqs], axis=-1)
    cos = np.cos(ang).T.astype(np.float32)  # [32, n]
    sin = np.sin(ang).T.astype(np.float32)
    return np.tile(cos, (4, 1)), np.tile(sin, (4, 1))


def host_prep(x, w_qkv, w_proj, n=4096, h=H, w=W):
    """Build the 8 per-core input maps."""
    x = np.asarray(x, dtype=np.float32)
    w_qkv = np.asarray(w_qkv, dtype=np.float32)
    w_proj = np.asarray(w_proj, dtype=np.float32)
    dim = x.shape[1]
    cos128, sin128 = rope_tables(h, w, n)

    def jmat(wh):  # wh [64, dim] -> J @ wh
        return np.concatenate([-wh[32:64], wh[0:32]], axis=0)

    in_maps = []
    for c in range(N_CORES):
        b, g = c // 2, c % 2
        hs = [4 * g + i for i in range(4)]
        cols = []
        for pair in range(2):
            h0, h1 = hs[2 * pair], hs[2 * pair + 1]
            wq0, wq1 = w_qkv[64 * h0:64 * h0 + 64], w_qkv[64 * h1:64 * h1 + 64]
            wk0 = w_qkv[dim + 64 * h0: dim + 64 * h0 + 64]
            wk1 = w_qkv[dim + 64 * h1: dim + 64 * h1 + 64]
            cols += [wq0, wq1, jmat(wq0), jmat(wq1),
                     wk0, wk1, jmat(wk0), jmat(wk1),
                     np.zeros((128, dim), np.float32)]  # v slot unused
        wqkvT = np.concatenate(cols, axis=0).T.copy()  # [dim, 1280]

        wvT = np.zeros((dim, 260), np.float32)
        for i, hh in enumerate(hs):
            wvT[:, 65 * i:65 * i + 64] = w_qkv[2 * dim + 64 * hh:
                                               2 * dim + 64 * hh + 64].T
        wprojT = w_proj[:, 256 * g:256 * g + 256].T.copy()  # [256, dim]

        in_maps.append({
            "x": np.ascontiguousarray(x[b].reshape(dim, n)).astype(np.float16),
            "wqkvT": np.ascontiguousarray(wqkvT).astype(np.float16),
            "wvT": wvT.astype(np.float16),
            "wprojT": np.ascontiguousarray(wprojT).astype(np.float16),
            "cos": cos128[:, :n].copy(),
            "sin": sin128[:, :n].copy(),
        })
    return in_maps


_NC_CACHE = {}


def kernel(x, w_qkv, w_proj, trace=False):
    key = "full"
    if key not in _NC_CACHE:
        _NC_CACHE[key] = build_nc(**FULL)
    nc = _NC_CACHE[key]
    in_maps = host_prep(x, w_qkv, w_proj)
    res = run_bass_kernel_spmd(nc, in_maps, list(range(N_CORES)), trace=trace)
    outs = [res.results[c]["out"] for c in range(N_CORES)]
    full = np.empty((B, DIM, H, W), np.float32)
    for b in range(B):
        full[b] = (outs[2 * b] + outs[2 * b + 1]).reshape(DIM, H, W)
    kernel.last_results = res
    return full
